# revision 14
# baseline (speedup 1.0000x reference)
"""Trainium2 Bass kernel for nn_BaseDependentAttentionLayer (GNN edge attention).

Strategy (8 NeuronCores):
  - Shard origin nodes contiguously: core r owns origins [1250r, 1250(r+1)).
    Host sorts edges by origin so segment-softmax/scatter are core-local.
  - LayerNorm affine + attention scale folded into QKV weights on host;
    MLP collapsed (W12 = W1@W2).
  - Phase A: LN (bf16) + QKV per 128-node window; k then v AllGathered
    (bf16, Shared-HBM output => local writes + barrier on real HW) into
    full [N, 512] tables; q kept local.
  - Phase B per 128-origin window: one-hot "transpose-select" matmuls
    (is_transpose=True with a one-hot-column moving matrix) broadcast
    q-rows to edge columns and exp-rows to 64-wide head blocks, writing
    bf16 directly to PSUM (validated on HW) — keeps DVE elementwise in 2x
    mode and avoids scalar-engine copies. Scores reduce per head via a
    one-hot hmask matmul; scatter-add + denominator are one-hot matmuls
    accumulating over the window in PSUM. Per-window tile counts.
  - Softmax without max-subtraction (scores are O(1), exp safe in fp32);
    normalization after the scatter.
"""

import sys

sys.path.insert(0, "/opt/trn_rl_repo")

import numpy as np
import ml_dtypes

bf16 = ml_dtypes.bfloat16

N, E, D, H = 10000, 160000, 512, 8
HD = D // H
SCALE = HD**-0.5
NCORES = 8
NPC = N // NCORES  # 1250 nodes/core
W = 10  # windows per core
WIN = 128  # origins per window
ET = 128  # edges per tile
EPS_LN = 1e-5
EPS_DEN = 1e-16


def _host_prep(origin, dest, ew):
    order = np.argsort(origin, kind="stable")
    o_s, d_s = origin[order], dest[order]
    core_of = o_s // NPC
    wloc = (o_s - core_of * NPC) // WIN
    counts = np.zeros((NCORES, W), np.int64)
    for r in range(NCORES):
        cm = core_of == r
        wl = wloc[cm]
        for w in range(W):
            counts[r, w] = int(np.sum(wl == w))
    Tw = [max(1, int(np.ceil(counts[:, w].max() / ET))) for w in range(W)]
    T = max(Tw)
    cofs = np.array([w * T * ET for w in range(W + 1)])
    NB = int(cofs[-1])
    percore = []
    for r in range(NCORES):
        dd_ = np.zeros(NB, np.int16)  # global dest node id
        ol = np.zeros(NB, np.int64)  # origin-local (0..127); pads 0
        vm = np.zeros(NB, bool)  # valid (non-pad)
        et = np.zeros((NB, H), np.float32)
        cm = core_of == r
        for w in range(W):
            m = cm & (wloc == w)
            cnt = int(m.sum())
            o = int(cofs[w])
            dd_[o:o + cnt] = d_s[m].astype(np.int16)
            ol[o:o + cnt] = o_s[m] - r * NPC - w * WIN
            vm[o:o + cnt] = True
            et[o:o + cnt] = ew[order[m]]
        percore.append(dict(dd=dd_, ol=ol, vm=vm, et=et))
    return percore, Tw, cofs


def _wrap_idx(idx_flat):
    """int16 [n] -> wrapped [128, n/16] layout for dma_gather (idx i at
    [i%16, i//16], replicated over the 8 Q7 partition groups)."""
    w = idx_flat.reshape(-1, 16).T  # [16, n/16]
    return np.tile(w, (8, 1)).astype(np.int16)


def _win_geometry(Tw):
    """Per-window halves (gather granularity) and 4-tile blocks."""
    geo = []
    for T in Tw:
        hn0 = min(T, ((T + 1) // 2 + 3) // 4 * 4)
        halves = [(0, hn0)] + ([(hn0, T - hn0)] if T > hn0 else [])
        blocks = []
        for h0, hn in halves:
            for b0 in range(h0, h0 + hn, 4):
                blocks.append((b0, min(4, h0 + hn - b0)))
        geo.append((halves, blocks))
    return geo


def _build_program(Tw, mock_ag=False):
    import concourse.bass as bass
    import concourse.bacc as bacc
    import concourse.mybir as mybir
    import concourse.tile as tile

    dt = mybir.dt
    Alu = mybir.AluOpType
    Act = mybir.ActivationFunctionType

    TM = max(Tw)
    NB = W * TM * ET
    cofs = [w * TM * ET for w in range(W + 1)]
    GEO = _win_geometry(Tw)

    nc = bacc.Bacc(
        "TRN2", target_bir_lowering=False, debug=False, num_devices=NCORES
    )

    # ---------------- I/O ----------------
    xsb_t = nc.dram_tensor("xsb", [W * 128, D], dt.bfloat16, kind="ExternalInput")
    xs_t = nc.dram_tensor("xs", [W * 128, D], dt.float32, kind="ExternalInput")
    wq_t = nc.dram_tensor("wq", [128, 4, D], dt.bfloat16, kind="ExternalInput")
    wk_t = nc.dram_tensor("wk", [128, 4, D], dt.bfloat16, kind="ExternalInput")
    wv_t = nc.dram_tensor("wv", [128, 4, D], dt.bfloat16, kind="ExternalInput")
    w12_t = nc.dram_tensor("w12", [128, 4, D], dt.bfloat16, kind="ExternalInput")
    bias_t = nc.dram_tensor("bias", [1, 4, D], dt.bfloat16, kind="ExternalInput")
    hmask_t = nc.dram_tensor("hmask", [128, 4, H], dt.bfloat16, kind="ExternalInput")
    m1w_t = nc.dram_tensor("m1w", [H, D], dt.bfloat16, kind="ExternalInput")
    ones_t = nc.dram_tensor("ones1", [1, 128], dt.bfloat16, kind="ExternalInput")
    ident_t = nc.dram_tensor("ident", [128, 128], dt.bfloat16, kind="ExternalInput")
    dw_t = nc.dram_tensor("dw", [128, NB // 16], dt.int16, kind="ExternalInput")
    st_t = nc.dram_tensor("st", [128, NB], dt.bfloat16, kind="ExternalInput")
    stt_t = nc.dram_tensor("stt", [128, NB], dt.bfloat16, kind="ExternalInput")
    ewt_t = nc.dram_tensor("ewt", [H, NB], dt.float32, kind="ExternalInput")
    out_t = nc.dram_tensor("out", [W * 128, D], dt.bfloat16, kind="ExternalOutput")

    with tile.TileContext(nc) as tc:
        with (
            tc.tile_pool(name="const", bufs=1) as cpool,
            tc.tile_pool(name="persist", bufs=1) as ppool,
            tc.tile_pool(name="dram", bufs=1, space="DRAM") as dpool,
        ):
            # constants
            wq = cpool.tile([128, 4, D], dt.bfloat16)
            wk = cpool.tile([128, 4, D], dt.bfloat16)
            wv = cpool.tile([128, 4, D], dt.bfloat16)
            w12 = cpool.tile([128, 4, D], dt.bfloat16)
            biases = cpool.tile([1, 4, D], dt.bfloat16)
            hmask = cpool.tile([128, 4, H], dt.bfloat16)
            m1w = cpool.tile([H, D], dt.bfloat16)
            ones1 = cpool.tile([1, 128], dt.bfloat16)
            ident = cpool.tile([128, 128], dt.bfloat16)
            dw = cpool.tile([128, NB // 16], dt.int16)
            for tl, tn in [
                (wq, wq_t), (wk, wk_t), (wv, wv_t), (w12, w12_t),
                (biases, bias_t), (hmask, hmask_t), (m1w, m1w_t),
                (ones1, ones_t), (ident, ident_t), (dw, dw_t),
            ]:
                nc.sync.dma_start(tl[:], tn.ap())

            # persistent activations
            q_sb = ppool.tile([128, W, D], dt.bfloat16)
            zT_all = ppool.tile([128, W, 4, 128], dt.bfloat16)
            values = ppool.tile([128, W, D], dt.bfloat16)
            vT = ppool.tile([128, 4, W, 128], dt.bfloat16)

            # collective buffers (Shared: all 8 cores on one device's HBM)
            k_in = dpool.tile([NPC, D], dt.bfloat16)
            v_in = dpool.tile([NPC, D], dt.bfloat16)
            k_full = dpool.tile([N, D], dt.bfloat16, addr_space="Shared")
            v_full = dpool.tile([N, D], dt.bfloat16, addr_space="Shared")

            # ---------------- Phase A: LN + QKV ----------------
            with (
                nc.named_scope("phaseA_qkv"),
                tc.tile_pool(name="pA", bufs=2) as pa,
                tc.tile_pool(name="psA", bufs=2, space="PSUM") as psa,
            ):
                for g in range(W):
                    xg = pa.tile([128, D], dt.bfloat16, tag="xg")
                    nc.sync.dma_start(xg[:], xsb_t.ap()[g * 128:(g + 1) * 128, :])
                    musum = pa.tile([128, 1], dt.float32, tag="musum")
                    nc.vector.tensor_reduce(musum[:], xg[:], mybir.AxisListType.X, Alu.add)
                    mu = pa.tile([128, 1], dt.float32, tag="mu")
                    nc.vector.tensor_scalar_mul(mu[:], musum[:], 1.0 / D)
                    xc = pa.tile([128, D], dt.bfloat16, tag="xc")
                    nc.vector.tensor_scalar(xc[:], xg[:], mu[:], None, Alu.subtract)
                    sq = pa.tile([128, D], dt.bfloat16, tag="sq")
                    vs = pa.tile([128, 1], dt.float32, tag="vs")
                    nc.vector.scalar_tensor_tensor(
                        sq[:], xc[:], 1.0, xc[:], Alu.bypass, Alu.mult, accum_out=vs[:]
                    )
                    vr = pa.tile([128, 1], dt.float32, tag="vr")
                    nc.vector.tensor_scalar(vr[:], vs[:], 1.0 / D, EPS_LN, Alu.mult, Alu.add)
                    sd = pa.tile([128, 1], dt.float32, tag="sd")
                    nc.scalar.sqrt(sd[:], vr[:])
                    rstd = pa.tile([128, 1], dt.float32, tag="rstd")
                    nc.vector.reciprocal(rstd[:], sd[:])
                    z = pa.tile([128, D], dt.bfloat16, tag="z")
                    nc.vector.tensor_scalar(z[:], xc[:], rstd[:], None, Alu.mult)
                    zT_ps = psa.tile([128, 4, 128], dt.bfloat16, tag="zT_ps")
                    for c in range(4):
                        nc.tensor.transpose(
                            zT_ps[:, c, :], z[:, c * 128:(c + 1) * 128], ident[:]
                        )
                    nc.vector.tensor_copy(zT_all[:, g, :, :], zT_ps[:])

                def proj(g, wt, bi, dst_sb, dram=None):
                    lo = g * 128
                    rows = min(128, NPC - lo)
                    ps = psa.tile([128, D], dt.float32, tag="qkv_ps")
                    for c in range(4):
                        nc.tensor.matmul(
                            ps[:], zT_all[:, g, c, :], wt[:, c, :],
                            start=(c == 0), stop=False,
                        )
                    nc.tensor.matmul(
                        ps[:], ones1[:], biases[:, bi, :], start=False, stop=True
                    )
                    if dram is None:
                        nc.scalar.copy(dst_sb[:], ps[:])
                    else:
                        kvt = pa.tile([128, D], dt.bfloat16, tag="kvt")
                        nc.scalar.copy(kvt[:], ps[:])
                        nc.sync.dma_start(dram[lo:lo + rows, :], kvt[:rows, :])

                for g in range(W):
                    proj(g, wk, 1, None, k_in)
                if mock_ag:
                    nc.sync.dma_start(k_full[0:NPC, :], k_in[:])
                else:
                    nc.gpsimd.collective_compute(
                        "AllGather", Alu.bypass,
                        replica_groups=[list(range(NCORES))],
                        ins=[k_in.opt()], outs=[k_full.opt()],
                    )
                for g in range(W):
                    proj(g, wv, 2, None, v_in)
                if mock_ag:
                    nc.sync.dma_start(v_full[0:NPC, :], v_in[:])
                else:
                    nc.gpsimd.collective_compute(
                        "AllGather", Alu.bypass,
                        replica_groups=[list(range(NCORES))],
                        ins=[v_in.opt()], outs=[v_full.opt()],
                    )
                for g in range(W):
                    proj(g, wq, 0, q_sb[:, g, :])

            # ---------------- Phase B: edge loop ----------------
            with (
                nc.named_scope("phaseB_edges"),
                tc.tile_pool(name="pB", bufs=2) as pb,
                tc.tile_pool(name="psSel", bufs=1, space="PSUM") as pssel,
                tc.tile_pool(name="psSc", bufs=1, space="PSUM") as pssc,
                tc.tile_pool(name="psAcc", bufs=1, space="PSUM") as psacc,
            ):
                ghalves = _win_geometry([TM])[0][0]
                for w in range(W):
                    T = Tw[w]
                    _, blocks = GEO[w]
                    co = cofs[w]
                    gh = []
                    for h0, hn in ghalves:
                        ni = hn * ET
                        c0 = (co + h0 * ET) // 16
                        kT = pb.tile([128, 4, ni], dt.bfloat16, tag=f"kT{h0 > 0}")
                        nc.gpsimd.dma_gather(
                            out_ap=kT[:], in_ap=k_full[:],
                            idxs_ap=dw[:, c0:c0 + ni // 16],
                            num_idxs=ni, num_idxs_reg=ni, elem_size=D,
                            transpose=True, single_packet=False,
                        )
                        vG = pb.tile([128, hn, D], dt.bfloat16, tag=f"vG{h0 > 0}")
                        nc.gpsimd.dma_gather(
                            out_ap=vG[:], in_ap=v_full[:],
                            idxs_ap=dw[:, c0:c0 + ni // 16],
                            num_idxs=ni, num_idxs_reg=ni, elem_size=D,
                            single_packet=False,
                        )
                        gh.append((kT, vG))

                    stw = pb.tile([128, TM * ET], dt.bfloat16, tag="stw")
                    nc.sync.dma_start(stw[:, :T * ET], st_t.ap()[:, co:co + T * ET])
                    sttw = pb.tile([128, TM * ET], dt.bfloat16, tag="sttw")
                    nc.sync.dma_start(sttw[:, :T * ET], stt_t.ap()[:, co:co + T * ET])
                    ewtw = pb.tile([H, TM * ET], dt.float32, tag="ewtw")
                    nc.sync.dma_start(ewtw[:, :T * ET], ewt_t.ap()[:, co:co + T * ET])

                    unnorm = psacc.tile([128, D], dt.float32, tag="unnorm")
                    denomB = psacc.tile([128, H], dt.float32, tag="denomB")

                    for t0, bt in blocks:
                        EB = bt * ET
                        ecol = t0 * ET
                        hf = 0 if t0 < ghalves[0][1] else 1
                        kT, vG = gh[hf]
                        h0 = ghalves[hf][0]
                        kcol = (t0 - h0) * ET
                        # q broadcast to edge cols: bf16 PSUM via select
                        qgT = pssel.tile([128, 4, 512], dt.bfloat16, tag="qgT")
                        for c in range(4):
                            nc.tensor.transpose(
                                qgT[:, c, :EB],
                                q_sb[:, w, c * 128:(c + 1) * 128],
                                stw[:, ecol:ecol + EB],
                            )
                        kq = pb.tile([128, 4, 512], dt.bfloat16, tag="kq")
                        nc.vector.tensor_tensor(
                            kq[:, :, :EB], kT[:, :, kcol:kcol + EB],
                            qgT[:, :, :EB], Alu.mult,
                        )
                        sc = pssc.tile([8, 512], dt.float32, tag="sc")
                        for c in range(4):
                            nc.tensor.matmul(
                                sc[:, :EB], hmask[:, c, :], kq[:, c, :EB],
                                start=(c == 0), stop=(c == 3),
                            )
                        ws = pb.tile([8, 512], dt.bfloat16, tag="ws")
                        nc.vector.tensor_tensor(
                            ws[:, :EB], sc[:, :EB], ewtw[:, ecol:ecol + EB], Alu.mult
                        )
                        ews = pb.tile([8, 512], dt.bfloat16, tag="ews")
                        nc.scalar.activation(ews[:, :EB], ws[:, :EB], Act.Exp)
                        # exp broadcast to 64-wide head blocks: bf16 PSUM
                        b_ps = pssel.tile([128, 4, D], dt.bfloat16, tag="b_ps")
                        for t in range(bt):
                            nc.tensor.transpose(
                                b_ps[:, t, :], ews[:, t * ET:(t + 1) * ET], m1w[:]
                            )
                        wvx = pb.tile([128, 4, D + H], dt.bfloat16, tag="wvx")
                        nc.vector.tensor_tensor(
                            wvx[:, :bt, :D], vG[:, t0 - h0:t0 - h0 + bt, :],
                            b_ps[:, :bt, :D], Alu.mult,
                        )
                        nc.scalar.copy(wvx[:, :bt, D:D + H], b_ps[:, :bt, ::HD])
                        for t in range(bt):
                            tt = t0 + t
                            stcol = tt * ET
                            nc.tensor.matmul(
                                unnorm[:], sttw[:, stcol:stcol + ET], wvx[:, t, :D],
                                start=(tt == 0), stop=(tt == T - 1),
                            )
                            nc.tensor.matmul(
                                denomB[:], sttw[:, stcol:stcol + ET], wvx[:, t, D:D + H],
                                start=(tt == 0), stop=(tt == T - 1),
                            )

                    # window epilogue: divide + transpose values
                    den8 = pb.tile([128, H], dt.float32, tag="den8")
                    nc.vector.tensor_scalar(den8[:], denomB[:], EPS_DEN, None, Alu.add)
                    rec8 = pb.tile([128, H], dt.float32, tag="rec8")
                    nc.vector.reciprocal(rec8[:], den8[:])
                    un_sb = pb.tile([128, D], dt.float32, tag="un_sb")
                    nc.vector.tensor_copy(un_sb[:], unnorm[:])
                    for h in range(H):
                        nc.gpsimd.tensor_scalar(
                            values[:, w, h * HD:(h + 1) * HD],
                            un_sb[:, h * HD:(h + 1) * HD],
                            rec8[:, h:h + 1], None, Alu.mult,
                        )
                    vt_ps = pssel.tile([128, 4, 128], dt.bfloat16, tag="vt_ps")
                    for c in range(4):
                        nc.tensor.transpose(
                            vt_ps[:, c, :], values[:, w, c * 128:(c + 1) * 128], ident[:]
                        )
                    nc.vector.tensor_copy(vT[:, :, w, :], vt_ps[:])

            # ---------------- Phase C: MLP + residual ----------------
            with (
                nc.named_scope("phaseC_mlp"),
                tc.tile_pool(name="pC", bufs=2) as pcl,
                tc.tile_pool(name="psC", bufs=2, space="PSUM") as psc,
            ):
                for g in range(W):
                    mlp_ps = psc.tile([128, D], dt.float32, tag="mlp")
                    for c in range(4):
                        nc.tensor.matmul(
                            mlp_ps[:], vT[:, c, g, :], w12[:, c, :],
                            start=(c == 0), stop=False,
                        )
                    nc.tensor.matmul(
                        mlp_ps[:], ones1[:], biases[:, 3, :], start=False, stop=True
                    )
                    xg2 = pcl.tile([128, D], dt.float32, tag="xg2")
                    nc.sync.dma_start(xg2[:], xs_t.ap()[g * 128:(g + 1) * 128, :])
                    og = pcl.tile([128, D], dt.bfloat16, tag="og")
                    nc.vector.tensor_tensor(og[:], mlp_ps[:], xg2[:], Alu.add)
                    nc.sync.dma_start(out_t.ap()[g * 128:(g + 1) * 128, :], og[:])

    nc.compile()
    from concourse.bass_interp import get_hw_module

    nc.m = get_hw_module(nc.m)
    return nc


def kernel(x, edge_index, edge_weights, ln_g, ln_b, Wq, bq, Wk, bk, Wv, bv,
           W1, b1, W2, b2, _trace=False):
    x = np.asarray(x, np.float32)
    ei = np.asarray(edge_index)
    ew = np.asarray(edge_weights, np.float32)
    origin, dest = ei[0].astype(np.int64), ei[1].astype(np.int64)

    percore, Tw, cofs = _host_prep(origin, dest, ew)

    # fold LN affine + attention scale into weights (host, fp32)
    ln_g = np.asarray(ln_g, np.float32)
    ln_b = np.asarray(ln_b, np.float32)
    Wq_f = (ln_g[:, None] * np.asarray(Wq, np.float32)) * SCALE
    bq_f = (ln_b @ np.asarray(Wq, np.float32)) * SCALE + np.asarray(bq, np.float32) * SCALE
    Wk_f = ln_g[:, None] * np.asarray(Wk, np.float32)
    bk_f = ln_b @ np.asarray(Wk, np.float32) + np.asarray(bk, np.float32)
    Wv_f = ln_g[:, None] * np.asarray(Wv, np.float32)
    bv_f = ln_b @ np.asarray(Wv, np.float32) + np.asarray(bv, np.float32)
    W12 = np.asarray(W1, np.float32) @ np.asarray(W2, np.float32)
    b12 = np.asarray(b1, np.float32) @ np.asarray(W2, np.float32) + np.asarray(b2, np.float32)

    def chunked(wm):  # [512, 512] -> [128, 4, 512]
        return np.ascontiguousarray(
            wm.reshape(4, 128, D).transpose(1, 0, 2)
        ).astype(bf16)

    hmask = np.zeros((128, 4, H), np.float32)
    for c in range(4):
        for d in range(128):
            hmask[d, c, (128 * c + d) // HD] = 1.0
    m1w = np.zeros((H, D), np.float32)
    for h in range(H):
        m1w[h, h * HD:(h + 1) * HD] = 1.0
    bias_all = np.stack([bq_f, bk_f, bv_f, b12])[None]  # [1, 4, 512]

    common = dict(
        wq=chunked(Wq_f), wk=chunked(Wk_f), wv=chunked(Wv_f), w12=chunked(W12),
        bias=bias_all.astype(bf16), hmask=hmask.astype(bf16), m1w=m1w.astype(bf16),
        ones1=np.ones((1, 128), bf16),
        ident=np.eye(128, dtype=bf16),
    )

    NB = int(cofs[-1])
    in_maps = []
    ar = np.arange(NB)
    tile_of = ar // ET
    e_in_tile = ar % ET
    for r in range(NCORES):
        pc = percore[r]
        ol = pc["ol"]
        vm = pc["vm"]
        st = np.zeros((128, NB), bf16)
        st[ol, ar] = 1
        stt = np.zeros((128, NB), bf16)
        stt[e_in_tile[vm], tile_of[vm] * ET + ol[vm]] = 1
        xsb = np.zeros((W * 128, D), np.float32)
        xsb[:NPC] = x[r * NPC:(r + 1) * NPC]
        in_maps.append(dict(
            xsb=xsb.astype(bf16),
            xs=xsb,
            dw=_wrap_idx(pc["dd"]),
            st=st, stt=stt,
            ewt=np.ascontiguousarray(pc["et"].T).astype(np.float32),
            **common,
        ))

    nc = _build_program(Tw)
    from concourse import bass_utils

    res = bass_utils.run_bass_kernel_spmd(
        nc, in_maps, core_ids=list(range(NCORES))
    )
    out = np.concatenate(
        [res.results[r]["out"][:NPC] for r in range(NCORES)], axis=0
    )
    kernel.last_result = res
    if _trace:
        import bench_hw

        kernel.exec_time_ns = bench_hw.bench(nc, in_maps, NCORES)
    return out.astype(np.float32)
